# revision 15
# baseline (speedup 1.0000x reference)
"""Cross-attention 1d kernel for Trainium2 (Bass/Tile), SPMD over 8 NeuronCores.

Problem (hardcoded shapes): N=4, C=512, L=2048, H=8, D=64.
  out_a = out_a_w @ attn(a_norm -> b_norm) + out_a_b + a
  out_b = out_b_w @ attn(b_norm -> a_norm) + out_b_b + b

Sharding: 8 cores = 4 samples x 2 directions (a->b, b->a). Each core computes
one full [512, 2048] output tensor; no cross-core communication.

Per-core dataflow (v2 — fp8e4 DoubleRow matmuls + 2-engine softmax exp):
  - All GEMMs use fp8e4 inputs with MatmulPerfMode.DoubleRow (2 contraction
    tiles per pass, 0.5 cycles/row): projections pair cin-tiles, scores pair
    two 32-wide halves of the head dim (q/k stored d-split via a host-side
    output-channel permutation of wq/wk), attn@V pairs adjacent kpos tiles.
  - Softmax: scores stay unscaled in PSUM f32; exp folds the 1/sqrt(D) scale.
    Exp tiles are split between the ACT engine (native Exp, fp8e4 output)
    and the DVE (Schraudolph fast-exp: one tensor_scalar writing the fp8e4
    bit pattern via uint8, using round-to-nearest f32->u8 conversion).
  - Softmax denominator comes free from 64 ones-columns appended to v
    (partitions 64:128 of the attn@V accumulator), normalize on DVE.
  - GroupNorm stats in f32: plain sums + affine applies on the Pool engine,
    Square+accum on ACT; normalized activations quantized to fp8e4.
  - Out-projection + bias + residual in f32 (exact residual path).
"""

import sys

sys.path.insert(0, "/opt/trn_rl_repo")

import numpy as np
import ml_dtypes

import concourse.bass as bass
import concourse.tile as tile
from concourse import bacc, mybir
from concourse.bass import ts
from concourse.bass_utils import run_bass_kernel_spmd

F32 = mybir.dt.float32
FP8 = mybir.dt.float8e4
U8 = mybir.dt.uint8
AF = mybir.ActivationFunctionType
ALU = mybir.AluOpType
PM = mybir.MatmulPerfMode

N, C, L, H = 4, 512, 2048, 8
D = C // H
EPS = 1e-5
SCALE = float(D) ** -0.5
P = 128
CO = C // P          # 4 channel chunks (standard order)
CH = 2               # q/k chunks (4 heads each, d-split layout)
LC = L // 512        # 4 column chunks of 512
LT = L // P          # 16 position tiles of 128
TP = LT // 2         # 8 kpos tile-pairs
QH = 4               # q processed in quarters of 512
QW = L // QH

# Schraudolph fast-exp to fp8e4 bits: bits = s*K + B (round-to-nearest)
#   value(bits) ~= 2^(bits/8 - 7)  =>  K = SCALE*log2(e)*8/8... folded:
#   score = s*SCALE; bits = score*log2(e)*8 + 56 + sigma
EXP_K = SCALE * 1.4426950408889634 * 8.0
EXP_B = 56.0 - 0.47

# kt indices whose exp runs on DVE (Schraudolph); rest on ACT
DVE_KT = frozenset({1, 4, 7, 10, 13})

FP8_NP = ml_dtypes.float8_e4m3


def _build_module():
    nc = bacc.Bacc("TRN2", target_bir_lowering=False, debug=False, num_devices=8)

    def din(name, shape, dt=F32):
        return nc.dram_tensor(name, list(shape), dt, kind="ExternalInput")

    x_d = din("x", (C, L))            # query-side input (residual side)
    y_d = din("y", (C, L))            # key/value-side input
    gnx_w = din("gnx_w", (C,))
    gnx_b = din("gnx_b", (C,))
    gny_w = din("gny_w", (C,))
    gny_b = din("gny_b", (C,))
    # weights, host-prepped to SBUF layout [p_cin, kk, slot, cout] fp8
    wq8_d = din("wq8", (P, 2, 2, C), FP8)   # couts permuted (chunk,dtile,i)
    wk8_d = din("wk8", (P, 2, 2, C), FP8)   # couts permuted
    wv8_d = din("wv8", (P, 2, 2, C), FP8)   # couts standard
    wo8_d = din("wo8", (P, 2, 2, C), FP8)   # couts standard
    bq_d = din("bq_p", (P, 2, 2))           # permuted, [i, chunk, dtile]
    bk_d = din("bk_p", (P, 2, 2))
    bv_d = din("bv", (C,))
    bo_d = din("bo_p", (P, CO))
    out_d = nc.dram_tensor("out", [C, L], F32, kind="ExternalOutput")

    inv_cnt = 1.0 / float(C * L)

    with tile.TileContext(nc) as tc:
        with (
            tc.tile_pool(name="persist", bufs=1) as pp,
            tc.tile_pool(name="small", bufs=1) as sp,
        ):
            # ---- persistent tiles ----
            q8 = pp.tile([P, CH, 2, L], FP8)         # d-split layout   8K
            k8 = pp.tile([P, CH, 2, L], FP8)         #                  8K
            vaug8 = pp.tile([P, TP, 2, H, P], FP8)   # [l, tp, sl, h, 64v|64one] 16K
            attn8 = pp.tile([P, CO, L], FP8)         # attention out [c,L] 8K
            wq8 = pp.tile([P, 2, 2, C], FP8)         # 2K each
            wk8 = pp.tile([P, 2, 2, C], FP8)
            wv8 = pp.tile([P, 2, 2, C], FP8)
            wo8 = pp.tile([P, 2, 2, C], FP8)

            ones_col = sp.tile([P, 1], F32)
            ones_row = sp.tile([1, P], F32)
            nc.vector.memset(ones_col[:], 1.0)
            nc.vector.memset(ones_row[:], 1.0)
            bq_pc = sp.tile([P, 2, 2], F32)
            bk_pc = sp.tile([P, 2, 2], F32)
            bo_pc = sp.tile([P, CO], F32)
            bv_row = sp.tile([1, C], F32)
            bv_bc = sp.tile([P, C], F32)
            gnw_y_pc = sp.tile([P, CO], F32)
            gnb_y_pc = sp.tile([P, CO], F32)
            gnw_x_pc = sp.tile([P, CO], F32)
            gnb_x_pc = sp.tile([P, CO], F32)
            # ones half of v_aug, set once
            nc.gpsimd.memset(vaug8[:, :, :, :, D:P], 1.0)

            with (
                tc.tile_pool(name="norm", bufs=1) as npool,
                tc.tile_pool(name="ps_mm", bufs=3, space="PSUM") as psM,
            ):
                yn8 = npool.tile([P, CO, L], FP8)
                xn8 = npool.tile([P, CO, L], FP8)

                with tc.tile_pool(name="gn_scr", bufs=2) as gsp:
                    def gn_scale_bias(src_sb, w_d, b_d, pref):
                        """[P,CO] scale/bias tiles: x_norm = x*scale + bias."""
                        st = sp.tile([P, 2], F32, tag=f"{pref}_st")
                        parts = gsp.tile([P, CO], F32, tag="gn_parts")
                        for co in range(CO):
                            nc.vector.tensor_reduce(parts[:, co:co + 1],
                                                    src_sb[:, co, :],
                                                    axis=mybir.AxisListType.X,
                                                    op=ALU.add)
                        nc.vector.tensor_reduce(st[:, 0:1], parts[:],
                                                axis=mybir.AxisListType.X,
                                                op=ALU.add)
                        sqp = gsp.tile([P, CO], F32, tag="gn_sqp")
                        for co in range(CO):
                            scr = gsp.tile([P, L], mybir.dt.bfloat16,
                                           tag="gn_scr")
                            nc.scalar.activation(scr[:], src_sb[:, co, :],
                                                 AF.Square,
                                                 accum_out=sqp[:, co:co + 1])
                        nc.vector.tensor_reduce(st[:, 1:2], sqp[:],
                                                axis=mybir.AxisListType.X,
                                                op=ALU.add)
                        # cross-partition reduce then broadcast back, via PE
                        tot_p = psM.tile([1, 2], F32, tag="mm")
                        nc.tensor.matmul(tot_p[:], ones_col[:], st[:],
                                         start=True, stop=True)
                        t12 = sp.tile([1, 2], F32, tag=f"{pref}_t12")
                        nc.scalar.copy(t12[:], tot_p[:])
                        bc_p = psM.tile([P, 2], F32, tag="mm")
                        nc.tensor.matmul(bc_p[:], ones_row[:], t12[:],
                                         start=True, stop=True)
                        tot = sp.tile([P, 2], F32, tag=f"{pref}_tot")
                        nc.vector.tensor_copy(tot[:], bc_p[:])

                        mu = sp.tile([P, 1], F32, tag=f"{pref}_mu")
                        nc.vector.tensor_scalar(mu[:], tot[:, 0:1], inv_cnt, 0.0,
                                                op0=ALU.mult, op1=ALU.add)
                        var = sp.tile([P, 1], F32, tag=f"{pref}_var")
                        nc.vector.tensor_scalar(var[:], tot[:, 1:2], inv_cnt, EPS,
                                                op0=ALU.mult, op1=ALU.add)
                        musq = sp.tile([P, 1], F32, tag=f"{pref}_musq")
                        nc.vector.tensor_scalar(musq[:], mu[:], mu[:], 0.0,
                                                op0=ALU.mult, op1=ALU.add)
                        nc.vector.tensor_tensor(var[:], var[:], musq[:],
                                                ALU.subtract)
                        std = sp.tile([P, 1], F32, tag=f"{pref}_std")
                        nc.scalar.activation(std[:], var[:], AF.Sqrt)
                        rstd = sp.tile([P, 1], F32, tag=f"{pref}_rstd")
                        nc.vector.reciprocal(rstd[:], std[:])
                        nmu = sp.tile([P, 1], F32, tag=f"{pref}_nmu")
                        nc.vector.tensor_scalar(nmu[:], mu[:], -1.0, 0.0,
                                                op0=ALU.mult, op1=ALU.add)

                        scale = sp.tile([P, CO], F32, tag=f"{pref}_scale")
                        bias = sp.tile([P, CO], F32, tag=f"{pref}_bias")
                        nc.vector.tensor_scalar(scale[:], w_d[:], rstd[:], 0.0,
                                                op0=ALU.mult, op1=ALU.add)
                        nc.vector.scalar_tensor_tensor(bias[:], scale[:], nmu[:],
                                                       b_d[:],
                                                       op0=ALU.mult, op1=ALU.add)
                        return scale, bias

                    with tc.tile_pool(name="ph_xy", bufs=1) as yp:
                        y_sb = yp.tile([P, CO, L], F32)
                        x_sb = yp.tile([P, CO, L], F32)
                        for co in range(CO):
                            nc.sync.dma_start(
                                y_sb[:, co, :],
                                y_d[:].rearrange("(co p) l -> p co l", p=P)[:, co, :])
                        for co in range(CO):
                            nc.sync.dma_start(
                                x_sb[:, co, :],
                                x_d[:].rearrange("(co p) l -> p co l", p=P)[:, co, :])
                        for dr, t in ((gny_w, gnw_y_pc), (gny_b, gnb_y_pc),
                                      (gnx_w, gnw_x_pc), (gnx_b, gnb_x_pc)):
                            nc.sync.dma_start(
                                t[:], dr[:].rearrange("(co p) -> p co", p=P))
                        nc.sync.dma_start(bq_pc[:], bq_d[:])
                        nc.sync.dma_start(bk_pc[:], bk_d[:])
                        nc.sync.dma_start(bo_pc[:], bo_d[:])
                        nc.sync.dma_start(
                            bv_row[:], bv_d[:].rearrange("(a c) -> a c", a=1))
                        nc.gpsimd.partition_broadcast(bv_bc[:], bv_row[:])
                        for dr, t in ((wv8_d, wv8), (wk8_d, wk8),
                                      (wq8_d, wq8), (wo8_d, wo8)):
                            nc.sync.dma_start(t[:], dr[:])
                        s_y, b_y = gn_scale_bias(y_sb, gnw_y_pc, gnb_y_pc, "y")
                        for co in range(CO):
                            nc.gpsimd.tensor_scalar(yn8[:, co, :], y_sb[:, co, :],
                                                    s_y[:, co:co + 1],
                                                    b_y[:, co:co + 1],
                                                    op0=ALU.mult, op1=ALU.add)
                        s_x, b_x = gn_scale_bias(x_sb, gnw_x_pc, gnb_x_pc, "x")
                        for co in range(CO):
                            nc.vector.tensor_scalar(xn8[:, co, :], x_sb[:, co, :],
                                                    s_x[:, co:co + 1],
                                                    b_x[:, co:co + 1],
                                                    op0=ALU.mult, op1=ALU.add)

                    # vT = (wv @ yn)^T + bv -> vaug8[:, lt//2, lt%2, h, 0:64]
                    for lt in range(LT):
                        vp = psM.tile([P, C], F32, tag="mm")
                        for kk in range(2):
                            nc.tensor.matmul(vp[:],
                                             yn8[:, 2 * kk:2 * kk + 2, ts(lt, P)],
                                             wv8[:, kk, :, :],
                                             start=(kk == 0), stop=(kk == 1),
                                             perf_mode=PM.DoubleRow)
                        nc.vector.tensor_tensor(
                            vaug8[:, lt // 2, lt % 2, :, 0:D],
                            vp[:].rearrange("p (h d) -> p h d", d=D),
                            bv_bc[:].rearrange("p (h d) -> p h d", d=D),
                            ALU.add)

                # ======== attention ========
                with (
                    tc.tile_pool(name="ps_out", bufs=1, space="PSUM") as ps_out,
                    tc.tile_pool(name="pt_pool", bufs=4) as ptp,
                    tc.tile_pool(name="tail", bufs=2) as tlp,
                ):
                    def qkv_mm(dst8, w8, src8, ch, dt_, bias_pc):
                        """dst8[:, ch, dt_, :] = w^T @ src + bias (DoubleRow)."""
                        m = ch * 2 + dt_
                        for lc in range(LC):
                            mmp = psM.tile([P, 512], F32, tag="mm")
                            for kk in range(2):
                                nc.tensor.matmul(
                                    mmp[:], w8[:, kk, :, ts(m, P)],
                                    src8[:, 2 * kk:2 * kk + 2, ts(lc, 512)],
                                    start=(kk == 0), stop=(kk == 1),
                                    perf_mode=PM.DoubleRow)
                            nc.scalar.activation(
                                dst8[:, ch, dt_, ts(lc, 512)], mmp[:],
                                AF.Identity,
                                bias=bias_pc[:, ch, dt_:dt_ + 1])

                    # Software-pipelined sweeps: attn@V lags the exp stream by
                    # two tile-pairs; each sweep's av(6..7) + softmax tail are
                    # emitted inside the NEXT sweep (keeps every engine's
                    # in-order queue from blocking on cross-engine waits).
                    state = {"f1": None, "f2": None}

                    def sweep(ch, p_, b, qq):
                        qs = qq * QW
                        p8ts = [None] * TP
                        box = {}

                        def av(tp, stop):
                            for hh in range(2):
                                nc.tensor.matmul(
                                    box["oAB"][:, hh, :],
                                    vaug8[:, tp, :, p_ * 2 + hh, :],
                                    p8ts[tp][:, :, hh, :],
                                    start=(tp == 0), stop=stop,
                                    perf_mode=PM.DoubleRow)

                        for tp in range(TP):
                            p8t = ptp.tile([P, 2, 2, QW], FP8, tag="pt")
                            p8ts[tp] = p8t
                            for sl in range(2):
                                kt = 2 * tp + sl
                                scp = psM.tile([P, 2, QW], F32, tag="mm")
                                for hh in range(2):
                                    bb = b + 32 * hh
                                    nc.tensor.matmul(
                                        scp[:, hh, :],
                                        k8[bb:bb + 32, ch, :, ts(kt, P)],
                                        q8[bb:bb + 32, ch, :, qs:qs + QW],
                                        start=True, stop=True,
                                        perf_mode=PM.DoubleRow,
                                        tile_position=(bb, 0))
                                if kt in DVE_KT:
                                    nc.vector.tensor_scalar(
                                        p8t[:, sl, :, :].bitcast(U8),
                                        scp[:], EXP_K, EXP_B,
                                        op0=ALU.mult, op1=ALU.add)
                                else:
                                    nc.scalar.activation(
                                        p8t[:, sl, :, :], scp[:],
                                        AF.Exp, scale=SCALE)
                            if tp == 1 and state["f1"] is not None:
                                state["f1"]()
                            if tp == 2:
                                if state["f2"] is not None:
                                    state["f2"]()
                                oAB = ps_out.tile([P, 2, QW], F32,
                                                  tag="oAB", name="oAB")
                                box["oAB"] = oAB
                            if tp >= 2:
                                av(tp - 2, stop=False)

                        def f1():
                            av(TP - 2, stop=False)
                            av(TP - 1, stop=True)
                            oAB = box["oAB"]
                            s_sb = tlp.tile([D, 2, QW], F32, tag="s")
                            nc.vector.tensor_copy(s_sb[:], oAB[D:P, :, :])
                            r_sb = tlp.tile([D, 2, QW], F32, tag="r")
                            scr = tlp.tile([D, 2, QW], F32, tag="rs")
                            nc.vector.reciprocal_approx_accurate(
                                r_sb[:], s_sb[:], scr[:])
                            box["r"] = r_sb

                        def f2():
                            oAB, r_sb = box["oAB"], box["r"]
                            for hh in range(2):
                                h = 2 * p_ + hh
                                lo = D * (h % 2)
                                nc.vector.tensor_tensor(
                                    attn8[lo:lo + D, h // 2, qs:qs + QW],
                                    oAB[0:D, hh, :], r_sb[:, hh, :],
                                    ALU.mult)

                        state["f1"], state["f2"] = f1, f2

                    for ch in range(CH):
                        for dt_ in range(2):
                            qkv_mm(k8, wk8, yn8, ch, dt_, bk_pc)
                            qkv_mm(q8, wq8, xn8, ch, dt_, bq_pc)
                        for pr in range(2):      # pairs 2*ch + pr
                            for qq in range(QH):
                                sweep(ch, 2 * ch + pr, 64 * pr, qq)
                    state["f1"]()
                    state["f2"]()

                    # ====== out projection + bias + residual (f32) ======
                    with (
                        tc.tile_pool(name="outsb", bufs=3) as osp,
                        tc.tile_pool(name="xre", bufs=3) as xrp,
                    ):
                        for lc in range(LC):
                            for mo in range(CO):
                                op = psM.tile([P, 512], F32, tag="mm")
                                for kk in range(2):
                                    nc.tensor.matmul(
                                        op[:], wo8[:, kk, :, ts(mo, P)],
                                        attn8[:, 2 * kk:2 * kk + 2, ts(lc, 512)],
                                        start=(kk == 0), stop=(kk == 1),
                                        perf_mode=PM.DoubleRow)
                                xr = xrp.tile([P, 512], F32, tag="xr")
                                nc.sync.dma_start(
                                    xr[:],
                                    x_d[:].rearrange("(mo p) l -> p mo l", p=P)[:, mo, ts(lc, 512)])
                                o_sb = osp.tile([P, 512], F32, tag="osb")
                                nc.vector.scalar_tensor_tensor(
                                    o_sb[:], op[:], bo_pc[:, mo:mo + 1], xr[:],
                                    op0=ALU.add, op1=ALU.add)
                                nc.sync.dma_start(
                                    out_d[:].rearrange("(mo p) l -> p mo l", p=P)[:, mo, ts(lc, 512)],
                                    o_sb[:])

    nc.compile()
    return nc


_NC_CACHE = None


def _get_module():
    global _NC_CACHE
    if _NC_CACHE is None:
        _NC_CACHE = _build_module()
    return _NC_CACHE


# output-channel permutation for q/k: chunk c holds heads 4c..4c+3;
# partition i of (chunk, dtile t) holds channel (4c + i//32)*64 + t*32 + i%32
_CH_IDX = np.empty((2, 2, P), np.int64)
for _c in range(2):
    for _t in range(2):
        _i = np.arange(P)
        _CH_IDX[_c, _t] = (4 * _c + _i // 32) * 64 + _t * 32 + _i % 32


def _w_dr(w):
    """[cout, cin] f32 -> [p_cin, kk, slot, cout] fp8 (cin = kk*256+slot*128+p)."""
    wt = np.ascontiguousarray(np.asarray(w, np.float32).T)     # [cin, cout]
    wt = wt.reshape(2, 2, P, C).transpose(2, 0, 1, 3)          # [p, kk, slot, cout]
    return np.ascontiguousarray(wt).astype(FP8_NP)


def _core_inputs(x, y, gnx_w, gnx_b, gny_w, gny_b, qw_q, qb_q, qw_kv, qb_kv, ow, ob):
    wq = np.asarray(qw_q[0:C], np.float32)
    wk = np.asarray(qw_kv[C:2 * C], np.float32)
    wv = np.asarray(qw_kv[2 * C:3 * C], np.float32)
    bq = np.asarray(qb_q[0:C], np.float32)
    bk = np.asarray(qb_kv[C:2 * C], np.float32)
    perm = _CH_IDX.transpose(2, 0, 1)                          # [p, chunk, dtile]
    return {
        "x": np.ascontiguousarray(x, dtype=np.float32),
        "y": np.ascontiguousarray(y, dtype=np.float32),
        "gnx_w": np.asarray(gnx_w, np.float32), "gnx_b": np.asarray(gnx_b, np.float32),
        "gny_w": np.asarray(gny_w, np.float32), "gny_b": np.asarray(gny_b, np.float32),
        "wq8": _w_dr(wq[_CH_IDX.reshape(-1)]),
        "wk8": _w_dr(wk[_CH_IDX.reshape(-1)]),
        "wv8": _w_dr(wv),
        "wo8": _w_dr(ow),
        "bq_p": np.ascontiguousarray(bq[perm]),
        "bk_p": np.ascontiguousarray(bk[perm]),
        "bv": np.asarray(qb_kv[2 * C:3 * C], np.float32),
        "bo_p": np.ascontiguousarray(
            np.asarray(ob, np.float32).reshape(CO, P).T),
    }


def kernel(a, b, gn_a_w, gn_a_b, gn_b_w, gn_b_b,
           qkv_a_w, qkv_a_b, qkv_b_w, qkv_b_b,
           out_a_w, out_a_b, out_b_w, out_b_b):
    a = np.asarray(a); b = np.asarray(b)
    nc = _get_module()
    in_maps = []
    for s in range(N):
        # direction a->b : q from a, k/v from b, output -> out_a[s]
        in_maps.append(_core_inputs(a[s], b[s], gn_a_w, gn_a_b, gn_b_w, gn_b_b,
                                    qkv_a_w, qkv_a_b, qkv_b_w, qkv_b_b,
                                    out_a_w, out_a_b))
        # direction b->a : q from b, k/v from a, output -> out_b[s]
        in_maps.append(_core_inputs(b[s], a[s], gn_b_w, gn_b_b, gn_a_w, gn_a_b,
                                    qkv_b_w, qkv_b_b, qkv_a_w, qkv_a_b,
                                    out_b_w, out_b_b))
    res = run_bass_kernel_spmd(nc, in_maps, core_ids=list(range(2 * N)))
    out_a = np.stack([res.results[2 * s]["out"] for s in range(N)])
    out_b = np.stack([res.results[2 * s + 1]["out"] for s in range(N)])
    return out_a.astype(np.float32), out_b.astype(np.float32)


# revision 16
# speedup vs baseline: 1.0387x; 1.0387x over previous
"""Cross-attention 1d kernel for Trainium2 (Bass/Tile), SPMD over 8 NeuronCores.

Problem (hardcoded shapes): N=4, C=512, L=2048, H=8, D=64.
  out_a = out_a_w @ attn(a_norm -> b_norm) + out_a_b + a
  out_b = out_b_w @ attn(b_norm -> a_norm) + out_b_b + b

Sharding: 8 cores = 4 samples x 2 directions (a->b, b->a). Each core computes
one full [512, 2048] output tensor; no cross-core communication.

Per-core dataflow (v2 — fp8e4 DoubleRow matmuls + 2-engine softmax exp):
  - All GEMMs use fp8e4 inputs with MatmulPerfMode.DoubleRow (2 contraction
    tiles per pass, 0.5 cycles/row): projections pair cin-tiles, scores pair
    two 32-wide halves of the head dim (q/k stored d-split via a host-side
    output-channel permutation of wq/wk), attn@V pairs adjacent kpos tiles.
  - Softmax: scores stay unscaled in PSUM f32; exp folds the 1/sqrt(D) scale.
    Exp tiles are split between the ACT engine (native Exp, fp8e4 output)
    and the DVE (Schraudolph fast-exp: one tensor_scalar writing the fp8e4
    bit pattern via uint8, using round-to-nearest f32->u8 conversion).
  - Softmax denominator comes free from 64 ones-columns appended to v
    (partitions 64:128 of the attn@V accumulator), normalize on DVE.
  - GroupNorm stats in f32: plain sums + affine applies on the Pool engine,
    Square+accum on ACT; normalized activations quantized to fp8e4.
  - Out-projection + bias + residual in f32 (exact residual path).
"""

import sys

sys.path.insert(0, "/opt/trn_rl_repo")

import numpy as np
import ml_dtypes

import concourse.bass as bass
import concourse.tile as tile
from concourse import bacc, mybir
from concourse.bass import ts
from concourse.bass_utils import run_bass_kernel_spmd

F32 = mybir.dt.float32
FP8 = mybir.dt.float8e4
U8 = mybir.dt.uint8
AF = mybir.ActivationFunctionType
ALU = mybir.AluOpType
PM = mybir.MatmulPerfMode

N, C, L, H = 4, 512, 2048, 8
D = C // H
EPS = 1e-5
SCALE = float(D) ** -0.5
P = 128
CO = C // P          # 4 channel chunks (standard order)
CH = 2               # q/k chunks (4 heads each, d-split layout)
LC = L // 512        # 4 column chunks of 512
LT = L // P          # 16 position tiles of 128
TP = LT // 2         # 8 kpos tile-pairs
QH = 4               # q processed in quarters of 512
QW = L // QH

# Schraudolph fast-exp to fp8e4 bits: bits = s*K + B (round-to-nearest)
#   value(bits) ~= 2^(bits/8 - 7)  =>  K = SCALE*log2(e)*8/8... folded:
#   score = s*SCALE; bits = score*log2(e)*8 + 56 + sigma
EXP_K = SCALE * 1.4426950408889634 * 8.0
EXP_B = 56.0 - 0.47

# kt indices whose exp runs on DVE (Schraudolph); rest on ACT
DVE_KT = frozenset({1, 5, 9, 13})

FP8_NP = ml_dtypes.float8_e4m3


def _build_module():
    nc = bacc.Bacc("TRN2", target_bir_lowering=False, debug=False, num_devices=8)

    def din(name, shape, dt=F32):
        return nc.dram_tensor(name, list(shape), dt, kind="ExternalInput")

    x_d = din("x", (C, L))            # query-side input (residual side)
    y_d = din("y", (C, L))            # key/value-side input
    gnx_w = din("gnx_w", (C,))
    gnx_b = din("gnx_b", (C,))
    gny_w = din("gny_w", (C,))
    gny_b = din("gny_b", (C,))
    # weights, host-prepped to SBUF layout [p_cin, kk, slot, cout] fp8
    wq8_d = din("wq8", (P, 2, 2, C), FP8)   # couts permuted (chunk,dtile,i)
    wk8_d = din("wk8", (P, 2, 2, C), FP8)   # couts permuted
    wv8_d = din("wv8", (P, 2, 2, C), FP8)   # couts standard
    wo8_d = din("wo8", (P, 2, 2, C), FP8)   # couts standard
    bq_d = din("bq_p", (P, 2, 2))           # permuted, [i, chunk, dtile]
    bk_d = din("bk_p", (P, 2, 2))
    bv_d = din("bv", (C,))
    bo_d = din("bo_p", (P, CO))
    out_d = nc.dram_tensor("out", [C, L], F32, kind="ExternalOutput")

    inv_cnt = 1.0 / float(C * L)

    with tile.TileContext(nc) as tc:
        with (
            tc.tile_pool(name="persist", bufs=1) as pp,
            tc.tile_pool(name="small", bufs=1) as sp,
        ):
            # ---- persistent tiles ----
            q8 = pp.tile([P, CH, 2, L], FP8)         # d-split layout   8K
            k8 = pp.tile([P, CH, 2, L], FP8)         #                  8K
            vaug8 = pp.tile([P, TP, 2, H, P], FP8)   # [l, tp, sl, h, 64v|64one] 16K
            attn8 = pp.tile([P, CO, L], FP8)         # attention out [c,L] 8K
            wq8 = pp.tile([P, 2, 2, C], FP8)         # 2K each
            wk8 = pp.tile([P, 2, 2, C], FP8)
            wv8 = pp.tile([P, 2, 2, C], FP8)
            wo8 = pp.tile([P, 2, 2, C], FP8)

            ones_col = sp.tile([P, 1], F32)
            ones_row = sp.tile([1, P], F32)
            nc.vector.memset(ones_col[:], 1.0)
            nc.vector.memset(ones_row[:], 1.0)
            bq_pc = sp.tile([P, 2, 2], F32)
            bk_pc = sp.tile([P, 2, 2], F32)
            bo_pc = sp.tile([P, CO], F32)
            bv_row = sp.tile([1, C], F32)
            bv_bc = sp.tile([P, C], F32)
            gnw_y_pc = sp.tile([P, CO], F32)
            gnb_y_pc = sp.tile([P, CO], F32)
            gnw_x_pc = sp.tile([P, CO], F32)
            gnb_x_pc = sp.tile([P, CO], F32)
            # ones half of v_aug, set once
            nc.gpsimd.memset(vaug8[:, :, :, :, D:P], 1.0)

            with (
                tc.tile_pool(name="norm", bufs=1) as npool,
                tc.tile_pool(name="ps_mm", bufs=3, space="PSUM") as psM,
            ):
                yn8 = npool.tile([P, CO, L], FP8)
                xn8 = npool.tile([P, CO, L], FP8)

                with tc.tile_pool(name="gn_scr", bufs=2) as gsp:
                    def gn_scale_bias(src_sb, w_d, b_d, pref):
                        """[P,CO] scale/bias tiles: x_norm = x*scale + bias."""
                        st = sp.tile([P, 2], F32, tag=f"{pref}_st")
                        parts = gsp.tile([P, CO], F32, tag="gn_parts")
                        for co in range(CO):
                            nc.vector.tensor_reduce(parts[:, co:co + 1],
                                                    src_sb[:, co, :],
                                                    axis=mybir.AxisListType.X,
                                                    op=ALU.add)
                        nc.vector.tensor_reduce(st[:, 0:1], parts[:],
                                                axis=mybir.AxisListType.X,
                                                op=ALU.add)
                        sqp = gsp.tile([P, CO], F32, tag="gn_sqp")
                        for co in range(CO):
                            scr = gsp.tile([P, L], mybir.dt.bfloat16,
                                           tag="gn_scr")
                            nc.scalar.activation(scr[:], src_sb[:, co, :],
                                                 AF.Square,
                                                 accum_out=sqp[:, co:co + 1])
                        nc.vector.tensor_reduce(st[:, 1:2], sqp[:],
                                                axis=mybir.AxisListType.X,
                                                op=ALU.add)
                        # cross-partition reduce then broadcast back, via PE
                        tot_p = psM.tile([1, 2], F32, tag="mm")
                        nc.tensor.matmul(tot_p[:], ones_col[:], st[:],
                                         start=True, stop=True)
                        t12 = sp.tile([1, 2], F32, tag=f"{pref}_t12")
                        nc.scalar.copy(t12[:], tot_p[:])
                        bc_p = psM.tile([P, 2], F32, tag="mm")
                        nc.tensor.matmul(bc_p[:], ones_row[:], t12[:],
                                         start=True, stop=True)
                        tot = sp.tile([P, 2], F32, tag=f"{pref}_tot")
                        nc.vector.tensor_copy(tot[:], bc_p[:])

                        mu = sp.tile([P, 1], F32, tag=f"{pref}_mu")
                        nc.vector.tensor_scalar(mu[:], tot[:, 0:1], inv_cnt, 0.0,
                                                op0=ALU.mult, op1=ALU.add)
                        var = sp.tile([P, 1], F32, tag=f"{pref}_var")
                        nc.vector.tensor_scalar(var[:], tot[:, 1:2], inv_cnt, EPS,
                                                op0=ALU.mult, op1=ALU.add)
                        musq = sp.tile([P, 1], F32, tag=f"{pref}_musq")
                        nc.vector.tensor_scalar(musq[:], mu[:], mu[:], 0.0,
                                                op0=ALU.mult, op1=ALU.add)
                        nc.vector.tensor_tensor(var[:], var[:], musq[:],
                                                ALU.subtract)
                        std = sp.tile([P, 1], F32, tag=f"{pref}_std")
                        nc.scalar.activation(std[:], var[:], AF.Sqrt)
                        rstd = sp.tile([P, 1], F32, tag=f"{pref}_rstd")
                        nc.vector.reciprocal(rstd[:], std[:])
                        nmu = sp.tile([P, 1], F32, tag=f"{pref}_nmu")
                        nc.vector.tensor_scalar(nmu[:], mu[:], -1.0, 0.0,
                                                op0=ALU.mult, op1=ALU.add)

                        scale = sp.tile([P, CO], F32, tag=f"{pref}_scale")
                        bias = sp.tile([P, CO], F32, tag=f"{pref}_bias")
                        nc.vector.tensor_scalar(scale[:], w_d[:], rstd[:], 0.0,
                                                op0=ALU.mult, op1=ALU.add)
                        nc.vector.scalar_tensor_tensor(bias[:], scale[:], nmu[:],
                                                       b_d[:],
                                                       op0=ALU.mult, op1=ALU.add)
                        return scale, bias

                    with tc.tile_pool(name="ph_xy", bufs=1) as yp:
                        y_sb = yp.tile([P, CO, L], F32)
                        x_sb = yp.tile([P, CO, L], F32)
                        for co in range(CO):
                            nc.sync.dma_start(
                                y_sb[:, co, :],
                                y_d[:].rearrange("(co p) l -> p co l", p=P)[:, co, :])
                        for co in range(CO):
                            nc.sync.dma_start(
                                x_sb[:, co, :],
                                x_d[:].rearrange("(co p) l -> p co l", p=P)[:, co, :])
                        for dr, t in ((gny_w, gnw_y_pc), (gny_b, gnb_y_pc),
                                      (gnx_w, gnw_x_pc), (gnx_b, gnb_x_pc)):
                            nc.sync.dma_start(
                                t[:], dr[:].rearrange("(co p) -> p co", p=P))
                        nc.sync.dma_start(bq_pc[:], bq_d[:])
                        nc.sync.dma_start(bk_pc[:], bk_d[:])
                        nc.sync.dma_start(bo_pc[:], bo_d[:])
                        nc.sync.dma_start(
                            bv_row[:], bv_d[:].rearrange("(a c) -> a c", a=1))
                        nc.gpsimd.partition_broadcast(bv_bc[:], bv_row[:])
                        for dr, t in ((wv8_d, wv8), (wk8_d, wk8),
                                      (wq8_d, wq8), (wo8_d, wo8)):
                            nc.sync.dma_start(t[:], dr[:])
                        s_y, b_y = gn_scale_bias(y_sb, gnw_y_pc, gnb_y_pc, "y")
                        for co in range(CO):
                            nc.gpsimd.tensor_scalar(yn8[:, co, :], y_sb[:, co, :],
                                                    s_y[:, co:co + 1],
                                                    b_y[:, co:co + 1],
                                                    op0=ALU.mult, op1=ALU.add)
                        s_x, b_x = gn_scale_bias(x_sb, gnw_x_pc, gnb_x_pc, "x")
                        for co in range(CO):
                            nc.vector.tensor_scalar(xn8[:, co, :], x_sb[:, co, :],
                                                    s_x[:, co:co + 1],
                                                    b_x[:, co:co + 1],
                                                    op0=ALU.mult, op1=ALU.add)

                    # vT = (wv @ yn)^T + bv -> vaug8[:, lt//2, lt%2, h, 0:64]
                    for lt in range(LT):
                        vp = psM.tile([P, C], F32, tag="mm")
                        for kk in range(2):
                            nc.tensor.matmul(vp[:],
                                             yn8[:, 2 * kk:2 * kk + 2, ts(lt, P)],
                                             wv8[:, kk, :, :],
                                             start=(kk == 0), stop=(kk == 1),
                                             perf_mode=PM.DoubleRow)
                        nc.vector.tensor_tensor(
                            vaug8[:, lt // 2, lt % 2, :, 0:D],
                            vp[:].rearrange("p (h d) -> p h d", d=D),
                            bv_bc[:].rearrange("p (h d) -> p h d", d=D),
                            ALU.add)

                # ======== attention ========
                with (
                    tc.tile_pool(name="ps_out", bufs=1, space="PSUM") as ps_out,
                    tc.tile_pool(name="pt_pool", bufs=4) as ptp,
                    tc.tile_pool(name="tail", bufs=2) as tlp,
                ):
                    def qkv_mm(dst8, w8, src8, ch, dt_, bias_pc):
                        """dst8[:, ch, dt_, :] = w^T @ src + bias (DoubleRow)."""
                        m = ch * 2 + dt_
                        for lc in range(LC):
                            mmp = psM.tile([P, 512], F32, tag="mm")
                            for kk in range(2):
                                nc.tensor.matmul(
                                    mmp[:], w8[:, kk, :, ts(m, P)],
                                    src8[:, 2 * kk:2 * kk + 2, ts(lc, 512)],
                                    start=(kk == 0), stop=(kk == 1),
                                    perf_mode=PM.DoubleRow)
                            nc.scalar.activation(
                                dst8[:, ch, dt_, ts(lc, 512)], mmp[:],
                                AF.Identity,
                                bias=bias_pc[:, ch, dt_:dt_ + 1])

                    # Software-pipelined sweeps: attn@V lags the exp stream by
                    # two tile-pairs; each sweep's av(6..7) + softmax tail are
                    # emitted inside the NEXT sweep (keeps every engine's
                    # in-order queue from blocking on cross-engine waits).
                    state = {"f1": None, "f2": None}

                    def sweep(ch, p_, b, qq):
                        qs = qq * QW
                        p8ts = [None] * TP
                        box = {}

                        def av(tp, stop):
                            for hh in range(2):
                                nc.tensor.matmul(
                                    box["oAB"][:, hh, :],
                                    vaug8[:, tp, :, p_ * 2 + hh, :],
                                    p8ts[tp][:, :, hh, :],
                                    start=(tp == 0), stop=stop,
                                    perf_mode=PM.DoubleRow)

                        for tp in range(TP):
                            p8t = ptp.tile([P, 2, 2, QW], FP8, tag="pt")
                            p8ts[tp] = p8t
                            for sl in range(2):
                                kt = 2 * tp + sl
                                scp = psM.tile([P, 2, QW], F32, tag="mm")
                                for hh in range(2):
                                    bb = b + 32 * hh
                                    nc.tensor.matmul(
                                        scp[:, hh, :],
                                        k8[bb:bb + 32, ch, :, ts(kt, P)],
                                        q8[bb:bb + 32, ch, :, qs:qs + QW],
                                        start=True, stop=True,
                                        perf_mode=PM.DoubleRow,
                                        tile_position=(bb, 0))
                                if kt in DVE_KT:
                                    nc.vector.tensor_scalar(
                                        p8t[:, sl, :, :].bitcast(U8),
                                        scp[:], EXP_K, EXP_B,
                                        op0=ALU.mult, op1=ALU.add)
                                else:
                                    nc.scalar.activation(
                                        p8t[:, sl, :, :], scp[:],
                                        AF.Exp, scale=SCALE)
                            if tp == 1 and state["f1"] is not None:
                                state["f1"]()
                            if tp == 2:
                                if state["f2"] is not None:
                                    state["f2"]()
                                oAB = ps_out.tile([P, 2, QW], F32,
                                                  tag="oAB", name="oAB")
                                box["oAB"] = oAB
                            if tp >= 2:
                                av(tp - 2, stop=False)

                        def f1():
                            av(TP - 2, stop=False)
                            av(TP - 1, stop=True)
                            oAB = box["oAB"]
                            s_sb = tlp.tile([D, 2, QW], F32, tag="s")
                            nc.vector.tensor_copy(s_sb[:], oAB[D:P, :, :])
                            r_sb = tlp.tile([D, 2, QW], F32, tag="r")
                            scr = tlp.tile([D, 2, QW], F32, tag="rs")
                            nc.vector.reciprocal_approx_accurate(
                                r_sb[:], s_sb[:], scr[:])
                            box["r"] = r_sb

                        def f2():
                            oAB, r_sb = box["oAB"], box["r"]
                            for hh in range(2):
                                h = 2 * p_ + hh
                                lo = D * (h % 2)
                                nc.vector.tensor_tensor(
                                    attn8[lo:lo + D, h // 2, qs:qs + QW],
                                    oAB[0:D, hh, :], r_sb[:, hh, :],
                                    ALU.mult)

                        state["f1"], state["f2"] = f1, f2

                    for ch in range(CH):
                        for dt_ in range(2):
                            qkv_mm(k8, wk8, yn8, ch, dt_, bk_pc)
                            qkv_mm(q8, wq8, xn8, ch, dt_, bq_pc)
                        for pr in range(2):      # pairs 2*ch + pr
                            for qq in range(QH):
                                sweep(ch, 2 * ch + pr, 64 * pr, qq)
                    state["f1"]()
                    state["f2"]()

                    # ====== out projection + bias + residual (f32) ======
                    with (
                        tc.tile_pool(name="outsb", bufs=3) as osp,
                        tc.tile_pool(name="xre", bufs=3) as xrp,
                    ):
                        for lc in range(LC):
                            for mo in range(CO):
                                op = psM.tile([P, 512], F32, tag="mm")
                                for kk in range(2):
                                    nc.tensor.matmul(
                                        op[:], wo8[:, kk, :, ts(mo, P)],
                                        attn8[:, 2 * kk:2 * kk + 2, ts(lc, 512)],
                                        start=(kk == 0), stop=(kk == 1),
                                        perf_mode=PM.DoubleRow)
                                xr = xrp.tile([P, 512], F32, tag="xr")
                                nc.sync.dma_start(
                                    xr[:],
                                    x_d[:].rearrange("(mo p) l -> p mo l", p=P)[:, mo, ts(lc, 512)])
                                o_sb = osp.tile([P, 512], F32, tag="osb")
                                nc.vector.scalar_tensor_tensor(
                                    o_sb[:], op[:], bo_pc[:, mo:mo + 1], xr[:],
                                    op0=ALU.add, op1=ALU.add)
                                nc.sync.dma_start(
                                    out_d[:].rearrange("(mo p) l -> p mo l", p=P)[:, mo, ts(lc, 512)],
                                    o_sb[:])

    nc.compile()
    return nc


_NC_CACHE = None


def _get_module():
    global _NC_CACHE
    if _NC_CACHE is None:
        _NC_CACHE = _build_module()
    return _NC_CACHE


# output-channel permutation for q/k: chunk c holds heads 4c..4c+3;
# partition i of (chunk, dtile t) holds channel (4c + i//32)*64 + t*32 + i%32
_CH_IDX = np.empty((2, 2, P), np.int64)
for _c in range(2):
    for _t in range(2):
        _i = np.arange(P)
        _CH_IDX[_c, _t] = (4 * _c + _i // 32) * 64 + _t * 32 + _i % 32


def _w_dr(w):
    """[cout, cin] f32 -> [p_cin, kk, slot, cout] fp8 (cin = kk*256+slot*128+p)."""
    wt = np.ascontiguousarray(np.asarray(w, np.float32).T)     # [cin, cout]
    wt = wt.reshape(2, 2, P, C).transpose(2, 0, 1, 3)          # [p, kk, slot, cout]
    return np.ascontiguousarray(wt).astype(FP8_NP)


def _core_inputs(x, y, gnx_w, gnx_b, gny_w, gny_b, qw_q, qb_q, qw_kv, qb_kv, ow, ob):
    wq = np.asarray(qw_q[0:C], np.float32)
    wk = np.asarray(qw_kv[C:2 * C], np.float32)
    wv = np.asarray(qw_kv[2 * C:3 * C], np.float32)
    bq = np.asarray(qb_q[0:C], np.float32)
    bk = np.asarray(qb_kv[C:2 * C], np.float32)
    perm = _CH_IDX.transpose(2, 0, 1)                          # [p, chunk, dtile]
    return {
        "x": np.ascontiguousarray(x, dtype=np.float32),
        "y": np.ascontiguousarray(y, dtype=np.float32),
        "gnx_w": np.asarray(gnx_w, np.float32), "gnx_b": np.asarray(gnx_b, np.float32),
        "gny_w": np.asarray(gny_w, np.float32), "gny_b": np.asarray(gny_b, np.float32),
        "wq8": _w_dr(wq[_CH_IDX.reshape(-1)]),
        "wk8": _w_dr(wk[_CH_IDX.reshape(-1)]),
        "wv8": _w_dr(wv),
        "wo8": _w_dr(ow),
        "bq_p": np.ascontiguousarray(bq[perm]),
        "bk_p": np.ascontiguousarray(bk[perm]),
        "bv": np.asarray(qb_kv[2 * C:3 * C], np.float32),
        "bo_p": np.ascontiguousarray(
            np.asarray(ob, np.float32).reshape(CO, P).T),
    }


def kernel(a, b, gn_a_w, gn_a_b, gn_b_w, gn_b_b,
           qkv_a_w, qkv_a_b, qkv_b_w, qkv_b_b,
           out_a_w, out_a_b, out_b_w, out_b_b):
    a = np.asarray(a); b = np.asarray(b)
    nc = _get_module()
    in_maps = []
    for s in range(N):
        # direction a->b : q from a, k/v from b, output -> out_a[s]
        in_maps.append(_core_inputs(a[s], b[s], gn_a_w, gn_a_b, gn_b_w, gn_b_b,
                                    qkv_a_w, qkv_a_b, qkv_b_w, qkv_b_b,
                                    out_a_w, out_a_b))
        # direction b->a : q from b, k/v from a, output -> out_b[s]
        in_maps.append(_core_inputs(b[s], a[s], gn_b_w, gn_b_b, gn_a_w, gn_a_b,
                                    qkv_b_w, qkv_b_b, qkv_a_w, qkv_a_b,
                                    out_b_w, out_b_b))
    res = run_bass_kernel_spmd(nc, in_maps, core_ids=list(range(2 * N)))
    out_a = np.stack([res.results[2 * s]["out"] for s in range(N)])
    out_b = np.stack([res.results[2 * s + 1]["out"] for s in range(N)])
    return out_a.astype(np.float32), out_b.astype(np.float32)


# revision 17
# speedup vs baseline: 1.0989x; 1.0580x over previous
"""Cross-attention 1d kernel for Trainium2 (Bass/Tile), SPMD over 8 NeuronCores.

Problem (hardcoded shapes): N=4, C=512, L=2048, H=8, D=64.
  out_a = out_a_w @ attn(a_norm -> b_norm) + out_a_b + a
  out_b = out_b_w @ attn(b_norm -> a_norm) + out_b_b + b

Sharding: 8 cores = 4 samples x 2 directions (a->b, b->a). Each core computes
one full [512, 2048] output tensor; no cross-core communication.

Per-core dataflow (v2 — fp8e4 DoubleRow matmuls + 2-engine softmax exp):
  - All GEMMs use fp8e4 inputs with MatmulPerfMode.DoubleRow (2 contraction
    tiles per pass, 0.5 cycles/row): projections pair cin-tiles, scores pair
    two 32-wide halves of the head dim (q/k stored d-split via a host-side
    output-channel permutation of wq/wk), attn@V pairs adjacent kpos tiles.
  - Softmax: scores stay unscaled in PSUM f32; exp folds the 1/sqrt(D) scale.
    Exp tiles are split between the ACT engine (native Exp, fp8e4 output)
    and the DVE (Schraudolph fast-exp: one tensor_scalar writing the fp8e4
    bit pattern via uint8, using round-to-nearest f32->u8 conversion).
  - Softmax denominator comes free from 64 ones-columns appended to v
    (partitions 64:128 of the attn@V accumulator), normalize on DVE.
  - GroupNorm stats in f32: plain sums + affine applies on the Pool engine,
    Square+accum on ACT; normalized activations quantized to fp8e4.
  - Out-projection + bias + residual in f32 (exact residual path).
"""

import sys

sys.path.insert(0, "/opt/trn_rl_repo")

import numpy as np
import ml_dtypes

import concourse.bass as bass
import concourse.tile as tile
from concourse import bacc, mybir
from concourse.bass import ts
from concourse.bass_utils import run_bass_kernel_spmd

F32 = mybir.dt.float32
FP8 = mybir.dt.float8e4
U8 = mybir.dt.uint8
AF = mybir.ActivationFunctionType
ALU = mybir.AluOpType
PM = mybir.MatmulPerfMode

N, C, L, H = 4, 512, 2048, 8
D = C // H
EPS = 1e-5
SCALE = float(D) ** -0.5
P = 128
CO = C // P          # 4 channel chunks (standard order)
CH = 2               # q/k chunks (4 heads each, d-split layout)
LC = L // 512        # 4 column chunks of 512
LT = L // P          # 16 position tiles of 128
TP = LT // 2         # 8 kpos tile-pairs
QH = 4               # q processed in quarters of 512
QW = L // QH

# Schraudolph fast-exp to fp8e4 bits: bits = s*K + B (round-to-nearest)
#   value(bits) ~= 2^(bits/8 - 7)  =>  K = SCALE*log2(e)*8/8... folded:
#   score = s*SCALE; bits = score*log2(e)*8 + 56 + sigma
EXP_K = SCALE * 1.4426950408889634 * 8.0
EXP_B = 56.0 - 0.47

# kt indices whose exp runs on DVE (Schraudolph); rest on ACT
DVE_KT = frozenset({1, 5, 9, 13})

FP8_NP = ml_dtypes.float8_e4m3


def _build_module():
    nc = bacc.Bacc("TRN2", target_bir_lowering=False, debug=False, num_devices=8)

    def din(name, shape, dt=F32):
        return nc.dram_tensor(name, list(shape), dt, kind="ExternalInput")

    x_d = din("x", (C, L))            # query-side input (residual side)
    y_d = din("y", (C, L))            # key/value-side input
    gnx_w = din("gnx_w", (C,))
    gnx_b = din("gnx_b", (C,))
    gny_w = din("gny_w", (C,))
    gny_b = din("gny_b", (C,))
    # weights, host-prepped to SBUF layout [p_cin, kk, slot, cout] fp8
    wq8_d = din("wq8", (P, 2, 2, C), FP8)   # couts permuted (chunk,dtile,i)
    wk8_d = din("wk8", (P, 2, 2, C), FP8)   # couts permuted
    wv8_d = din("wv8", (P, 2, 2, C), FP8)   # couts standard
    wo8_d = din("wo8", (P, 2, 2, C), FP8)   # couts standard
    bq_d = din("bq_p", (P, 2, 2))           # permuted, [i, chunk, dtile]
    bk_d = din("bk_p", (P, 2, 2))
    bv_d = din("bv", (C,))
    bo_d = din("bo_p", (P, CO))
    out_d = nc.dram_tensor("out", [C, L], F32, kind="ExternalOutput")

    inv_cnt = 1.0 / float(C * L)

    with tile.TileContext(nc) as tc:
        with (
            tc.tile_pool(name="persist", bufs=1) as pp,
            tc.tile_pool(name="small", bufs=1) as sp,
        ):
            # ---- persistent tiles ----
            q8 = pp.tile([P, CH, 2, L], FP8)         # d-split layout   8K
            k8 = pp.tile([P, CH, 2, L], FP8)         #                  8K
            vaug8 = pp.tile([P, TP, 2, H, P], FP8)   # [l, tp, sl, h, 64v|64one] 16K
            attn8 = pp.tile([P, CO, L], FP8)         # attention out [c,L] 8K
            wq8 = pp.tile([P, 2, 2, C], FP8)         # 2K each
            wk8 = pp.tile([P, 2, 2, C], FP8)
            wv8 = pp.tile([P, 2, 2, C], FP8)
            wo8 = pp.tile([P, 2, 2, C], FP8)

            ones_col = sp.tile([P, 1], F32)
            ones_row = sp.tile([1, P], F32)
            nc.vector.memset(ones_col[:], 1.0)
            nc.vector.memset(ones_row[:], 1.0)
            bq_pc = sp.tile([P, 2, 2], F32)
            bk_pc = sp.tile([P, 2, 2], F32)
            bo_pc = sp.tile([P, CO], F32)
            bv_row = sp.tile([1, C], F32)
            bv_bc = sp.tile([P, C], F32)
            gnw_y_pc = sp.tile([P, CO], F32)
            gnb_y_pc = sp.tile([P, CO], F32)
            gnw_x_pc = sp.tile([P, CO], F32)
            gnb_x_pc = sp.tile([P, CO], F32)
            # ones half of v_aug, set once
            nc.gpsimd.memset(vaug8[:, :, :, :, D:P], 1.0)

            with (
                tc.tile_pool(name="norm", bufs=1) as npool,
                tc.tile_pool(name="ps_mm", bufs=3, space="PSUM") as psM,
            ):
                yn8 = npool.tile([P, CO, L], FP8)
                xn8 = npool.tile([P, CO, L], FP8)

                with tc.tile_pool(name="gn_scr", bufs=2) as gsp:
                    def gn_scale_bias(src_sb, w_d, b_d, pref):
                        """[P,CO] scale/bias tiles: x_norm = x*scale + bias."""
                        st = sp.tile([P, 2], F32, tag=f"{pref}_st")
                        parts = gsp.tile([P, CO], F32, tag="gn_parts")
                        for co in range(CO):
                            nc.vector.tensor_reduce(parts[:, co:co + 1],
                                                    src_sb[:, co, :],
                                                    axis=mybir.AxisListType.X,
                                                    op=ALU.add)
                        nc.vector.tensor_reduce(st[:, 0:1], parts[:],
                                                axis=mybir.AxisListType.X,
                                                op=ALU.add)
                        sqp = gsp.tile([P, CO], F32, tag="gn_sqp")
                        for co in range(CO):
                            scr = gsp.tile([P, L], mybir.dt.bfloat16,
                                           tag="gn_scr")
                            nc.scalar.activation(scr[:], src_sb[:, co, :],
                                                 AF.Square,
                                                 accum_out=sqp[:, co:co + 1])
                        nc.vector.tensor_reduce(st[:, 1:2], sqp[:],
                                                axis=mybir.AxisListType.X,
                                                op=ALU.add)
                        # cross-partition reduce then broadcast back, via PE
                        tot_p = psM.tile([1, 2], F32, tag="mm")
                        nc.tensor.matmul(tot_p[:], ones_col[:], st[:],
                                         start=True, stop=True)
                        t12 = sp.tile([1, 2], F32, tag=f"{pref}_t12")
                        nc.scalar.copy(t12[:], tot_p[:])
                        bc_p = psM.tile([P, 2], F32, tag="mm")
                        nc.tensor.matmul(bc_p[:], ones_row[:], t12[:],
                                         start=True, stop=True)
                        tot = sp.tile([P, 2], F32, tag=f"{pref}_tot")
                        nc.vector.tensor_copy(tot[:], bc_p[:])

                        mu = sp.tile([P, 1], F32, tag=f"{pref}_mu")
                        nc.vector.tensor_scalar(mu[:], tot[:, 0:1], inv_cnt, 0.0,
                                                op0=ALU.mult, op1=ALU.add)
                        var = sp.tile([P, 1], F32, tag=f"{pref}_var")
                        nc.vector.tensor_scalar(var[:], tot[:, 1:2], inv_cnt, EPS,
                                                op0=ALU.mult, op1=ALU.add)
                        musq = sp.tile([P, 1], F32, tag=f"{pref}_musq")
                        nc.vector.tensor_scalar(musq[:], mu[:], mu[:], 0.0,
                                                op0=ALU.mult, op1=ALU.add)
                        nc.vector.tensor_tensor(var[:], var[:], musq[:],
                                                ALU.subtract)
                        std = sp.tile([P, 1], F32, tag=f"{pref}_std")
                        nc.scalar.activation(std[:], var[:], AF.Sqrt)
                        rstd = sp.tile([P, 1], F32, tag=f"{pref}_rstd")
                        nc.vector.reciprocal(rstd[:], std[:])
                        nmu = sp.tile([P, 1], F32, tag=f"{pref}_nmu")
                        nc.vector.tensor_scalar(nmu[:], mu[:], -1.0, 0.0,
                                                op0=ALU.mult, op1=ALU.add)

                        scale = sp.tile([P, CO], F32, tag=f"{pref}_scale")
                        bias = sp.tile([P, CO], F32, tag=f"{pref}_bias")
                        nc.vector.tensor_scalar(scale[:], w_d[:], rstd[:], 0.0,
                                                op0=ALU.mult, op1=ALU.add)
                        nc.vector.scalar_tensor_tensor(bias[:], scale[:], nmu[:],
                                                       b_d[:],
                                                       op0=ALU.mult, op1=ALU.add)
                        return scale, bias

                    with tc.tile_pool(name="ph_xy", bufs=1) as yp:
                        y_sb = yp.tile([P, CO, L], F32)
                        x_sb = yp.tile([P, CO, L], F32)
                        for co in range(CO):
                            nc.sync.dma_start(
                                y_sb[:, co, :],
                                y_d[:].rearrange("(co p) l -> p co l", p=P)[:, co, :])
                        for dr, t in ((gny_w, gnw_y_pc), (gny_b, gnb_y_pc),
                                      (gnx_w, gnw_x_pc), (gnx_b, gnb_x_pc)):
                            nc.sync.dma_start(
                                t[:], dr[:].rearrange("(co p) -> p co", p=P))
                        nc.sync.dma_start(bq_pc[:], bq_d[:])
                        nc.sync.dma_start(bk_pc[:], bk_d[:])
                        nc.sync.dma_start(bo_pc[:], bo_d[:])
                        nc.sync.dma_start(
                            bv_row[:], bv_d[:].rearrange("(a c) -> a c", a=1))
                        nc.gpsimd.partition_broadcast(bv_bc[:], bv_row[:])
                        for dr, t in ((wv8_d, wv8), (wk8_d, wk8),
                                      (wq8_d, wq8), (wo8_d, wo8)):
                            nc.sync.dma_start(t[:], dr[:])
                        for co in range(CO):
                            nc.sync.dma_start(
                                x_sb[:, co, :],
                                x_d[:].rearrange("(co p) l -> p co l", p=P)[:, co, :])
                        s_y, b_y = gn_scale_bias(y_sb, gnw_y_pc, gnb_y_pc, "y")
                        for co in range(CO):
                            nc.gpsimd.tensor_scalar(yn8[:, co, :], y_sb[:, co, :],
                                                    s_y[:, co:co + 1],
                                                    b_y[:, co:co + 1],
                                                    op0=ALU.mult, op1=ALU.add)
                        s_x, b_x = gn_scale_bias(x_sb, gnw_x_pc, gnb_x_pc, "x")
                        for co in range(CO):
                            nc.vector.tensor_scalar(xn8[:, co, :], x_sb[:, co, :],
                                                    s_x[:, co:co + 1],
                                                    b_x[:, co:co + 1],
                                                    op0=ALU.mult, op1=ALU.add)

                    # vT = (wv @ yn)^T + bv -> vaug8[:, lt//2, lt%2, h, 0:64]
                    for lt in range(LT):
                        vp = psM.tile([P, C], F32, tag="mm")
                        for kk in range(2):
                            nc.tensor.matmul(vp[:],
                                             yn8[:, 2 * kk:2 * kk + 2, ts(lt, P)],
                                             wv8[:, kk, :, :],
                                             start=(kk == 0), stop=(kk == 1),
                                             perf_mode=PM.DoubleRow)
                        nc.vector.tensor_tensor(
                            vaug8[:, lt // 2, lt % 2, :, 0:D],
                            vp[:].rearrange("p (h d) -> p h d", d=D),
                            bv_bc[:].rearrange("p (h d) -> p h d", d=D),
                            ALU.add)

                # ======== attention ========
                with (
                    tc.tile_pool(name="ps_out", bufs=1, space="PSUM") as ps_out,
                    tc.tile_pool(name="pt_pool", bufs=4) as ptp,
                    tc.tile_pool(name="tail", bufs=2) as tlp,
                ):
                    def qkv_mm(dst8, w8, src8, ch, dt_, bias_pc):
                        """dst8[:, ch, dt_, :] = w^T @ src + bias (DoubleRow)."""
                        m = ch * 2 + dt_
                        for lc in range(LC):
                            mmp = psM.tile([P, 512], F32, tag="mm")
                            for kk in range(2):
                                nc.tensor.matmul(
                                    mmp[:], w8[:, kk, :, ts(m, P)],
                                    src8[:, 2 * kk:2 * kk + 2, ts(lc, 512)],
                                    start=(kk == 0), stop=(kk == 1),
                                    perf_mode=PM.DoubleRow)
                            nc.scalar.activation(
                                dst8[:, ch, dt_, ts(lc, 512)], mmp[:],
                                AF.Identity,
                                bias=bias_pc[:, ch, dt_:dt_ + 1])

                    # Software-pipelined sweeps: attn@V lags the exp stream by
                    # two tile-pairs; each sweep's av(6..7) + softmax tail are
                    # emitted inside the NEXT sweep (keeps every engine's
                    # in-order queue from blocking on cross-engine waits).
                    state = {"f1": None, "f2": None}

                    def sweep(ch, p_, b, qq):
                        qs = qq * QW
                        p8ts = [None] * TP
                        box = {}

                        def av(tp, stop):
                            for hh in range(2):
                                nc.tensor.matmul(
                                    box["oAB"][:, hh, :],
                                    vaug8[:, tp, :, p_ * 2 + hh, :],
                                    p8ts[tp][:, :, hh, :],
                                    start=(tp == 0), stop=stop,
                                    perf_mode=PM.DoubleRow)

                        for tp in range(TP):
                            p8t = ptp.tile([P, 2, 2, QW], FP8, tag="pt")
                            p8ts[tp] = p8t
                            for sl in range(2):
                                kt = 2 * tp + sl
                                scp = psM.tile([P, 2, QW], F32, tag="mm")
                                for hh in range(2):
                                    bb = b + 32 * hh
                                    nc.tensor.matmul(
                                        scp[:, hh, :],
                                        k8[bb:bb + 32, ch, :, ts(kt, P)],
                                        q8[bb:bb + 32, ch, :, qs:qs + QW],
                                        start=True, stop=True,
                                        perf_mode=PM.DoubleRow,
                                        tile_position=(bb, 0))
                                if kt in DVE_KT:
                                    nc.vector.tensor_scalar(
                                        p8t[:, sl, :, :].bitcast(U8),
                                        scp[:], EXP_K, EXP_B,
                                        op0=ALU.mult, op1=ALU.add)
                                else:
                                    nc.scalar.activation(
                                        p8t[:, sl, :, :], scp[:],
                                        AF.Exp, scale=SCALE)
                            if tp == 1 and state["f1"] is not None:
                                state["f1"]()
                            if tp == 2:
                                if state["f2"] is not None:
                                    state["f2"]()
                                oAB = ps_out.tile([P, 2, QW], F32,
                                                  tag="oAB", name="oAB")
                                box["oAB"] = oAB
                            if tp >= 2:
                                av(tp - 2, stop=False)

                        def f1():
                            av(TP - 2, stop=False)
                            av(TP - 1, stop=True)
                            oAB = box["oAB"]
                            s_sb = tlp.tile([D, 2, QW], F32, tag="s")
                            nc.vector.tensor_copy(s_sb[:], oAB[D:P, :, :])
                            r_sb = tlp.tile([D, 2, QW], F32, tag="r")
                            nc.vector.reciprocal_approx_fast(r_sb[:], s_sb[:])
                            box["r"] = r_sb

                        def f2():
                            oAB, r_sb = box["oAB"], box["r"]
                            for hh in range(2):
                                h = 2 * p_ + hh
                                lo = D * (h % 2)
                                nc.vector.tensor_tensor(
                                    attn8[lo:lo + D, h // 2, qs:qs + QW],
                                    oAB[0:D, hh, :], r_sb[:, hh, :],
                                    ALU.mult)

                        state["f1"], state["f2"] = f1, f2

                    for ch in range(CH):
                        for dt_ in range(2):
                            qkv_mm(k8, wk8, yn8, ch, dt_, bk_pc)
                            qkv_mm(q8, wq8, xn8, ch, dt_, bq_pc)
                        for pr in range(2):      # pairs 2*ch + pr
                            for qq in range(QH):
                                sweep(ch, 2 * ch + pr, 64 * pr, qq)
                    state["f1"]()
                    state["f2"]()

                    # ====== out projection + bias + residual (f32) ======
                    with (
                        tc.tile_pool(name="outsb", bufs=3) as osp,
                        tc.tile_pool(name="xre", bufs=3) as xrp,
                    ):
                        for lc in range(LC):
                            for mo in range(CO):
                                op = psM.tile([P, 512], F32, tag="mm")
                                for kk in range(2):
                                    nc.tensor.matmul(
                                        op[:], wo8[:, kk, :, ts(mo, P)],
                                        attn8[:, 2 * kk:2 * kk + 2, ts(lc, 512)],
                                        start=(kk == 0), stop=(kk == 1),
                                        perf_mode=PM.DoubleRow)
                                xr = xrp.tile([P, 512], F32, tag="xr")
                                nc.sync.dma_start(
                                    xr[:],
                                    x_d[:].rearrange("(mo p) l -> p mo l", p=P)[:, mo, ts(lc, 512)])
                                o_sb = osp.tile([P, 512], F32, tag="osb")
                                nc.vector.scalar_tensor_tensor(
                                    o_sb[:], op[:], bo_pc[:, mo:mo + 1], xr[:],
                                    op0=ALU.add, op1=ALU.add)
                                nc.sync.dma_start(
                                    out_d[:].rearrange("(mo p) l -> p mo l", p=P)[:, mo, ts(lc, 512)],
                                    o_sb[:])

    nc.compile()
    return nc


_NC_CACHE = None


def _get_module():
    global _NC_CACHE
    if _NC_CACHE is None:
        _NC_CACHE = _build_module()
    return _NC_CACHE


# output-channel permutation for q/k: chunk c holds heads 4c..4c+3;
# partition i of (chunk, dtile t) holds channel (4c + i//32)*64 + t*32 + i%32
_CH_IDX = np.empty((2, 2, P), np.int64)
for _c in range(2):
    for _t in range(2):
        _i = np.arange(P)
        _CH_IDX[_c, _t] = (4 * _c + _i // 32) * 64 + _t * 32 + _i % 32


def _w_dr(w):
    """[cout, cin] f32 -> [p_cin, kk, slot, cout] fp8 (cin = kk*256+slot*128+p)."""
    wt = np.ascontiguousarray(np.asarray(w, np.float32).T)     # [cin, cout]
    wt = wt.reshape(2, 2, P, C).transpose(2, 0, 1, 3)          # [p, kk, slot, cout]
    return np.ascontiguousarray(wt).astype(FP8_NP)


def _core_inputs(x, y, gnx_w, gnx_b, gny_w, gny_b, qw_q, qb_q, qw_kv, qb_kv, ow, ob):
    wq = np.asarray(qw_q[0:C], np.float32)
    wk = np.asarray(qw_kv[C:2 * C], np.float32)
    wv = np.asarray(qw_kv[2 * C:3 * C], np.float32)
    bq = np.asarray(qb_q[0:C], np.float32)
    bk = np.asarray(qb_kv[C:2 * C], np.float32)
    perm = _CH_IDX.transpose(2, 0, 1)                          # [p, chunk, dtile]
    return {
        "x": np.ascontiguousarray(x, dtype=np.float32),
        "y": np.ascontiguousarray(y, dtype=np.float32),
        "gnx_w": np.asarray(gnx_w, np.float32), "gnx_b": np.asarray(gnx_b, np.float32),
        "gny_w": np.asarray(gny_w, np.float32), "gny_b": np.asarray(gny_b, np.float32),
        "wq8": _w_dr(wq[_CH_IDX.reshape(-1)]),
        "wk8": _w_dr(wk[_CH_IDX.reshape(-1)]),
        "wv8": _w_dr(wv),
        "wo8": _w_dr(ow),
        "bq_p": np.ascontiguousarray(bq[perm]),
        "bk_p": np.ascontiguousarray(bk[perm]),
        "bv": np.asarray(qb_kv[2 * C:3 * C], np.float32),
        "bo_p": np.ascontiguousarray(
            np.asarray(ob, np.float32).reshape(CO, P).T),
    }


def kernel(a, b, gn_a_w, gn_a_b, gn_b_w, gn_b_b,
           qkv_a_w, qkv_a_b, qkv_b_w, qkv_b_b,
           out_a_w, out_a_b, out_b_w, out_b_b):
    a = np.asarray(a); b = np.asarray(b)
    nc = _get_module()
    in_maps = []
    for s in range(N):
        # direction a->b : q from a, k/v from b, output -> out_a[s]
        in_maps.append(_core_inputs(a[s], b[s], gn_a_w, gn_a_b, gn_b_w, gn_b_b,
                                    qkv_a_w, qkv_a_b, qkv_b_w, qkv_b_b,
                                    out_a_w, out_a_b))
        # direction b->a : q from b, k/v from a, output -> out_b[s]
        in_maps.append(_core_inputs(b[s], a[s], gn_b_w, gn_b_b, gn_a_w, gn_a_b,
                                    qkv_b_w, qkv_b_b, qkv_a_w, qkv_a_b,
                                    out_b_w, out_b_b))
    res = run_bass_kernel_spmd(nc, in_maps, core_ids=list(range(2 * N)))
    out_a = np.stack([res.results[2 * s]["out"] for s in range(N)])
    out_b = np.stack([res.results[2 * s + 1]["out"] for s in range(N)])
    return out_a.astype(np.float32), out_b.astype(np.float32)
